# revision 12
# baseline (speedup 1.0000x reference)
"""GTMGC transformer block on 8 Trainium2 NeuronCores, pure data-parallel over batch.

Reference computation (per molecule, N=128 nodes, d=256, 8 heads, ffn 1024):
  Q/K/V = X@W + b ; scores = QK^T/sqrt(32) + adj + dist + maskbias
  attn = softmax(scores) ; ctx = attn@V ; X1 = X + LN(ctx@Wo + bo)
  X2 = X1 + LN(gelu(X1@W1 + b1)@W2 + b2) ; returns (X2, attn)

Device strategy per core (B/8 = 32 molecules, python-unrolled):
  - canonical transposed activations (feature dim on partitions) so all
    matmuls contract over partitions; PE transposes via identity matmul
  - matmuls in bf16 (weights host-precast), everything else f32
  - host folds: 1/sqrt(32) into Wq/bq; bv into bo_eff = bv@Wo+bo (valid
    because softmax rows sum to 1); adj+dist+mask into one bias matrix
"""

import math
import numpy as np
import ml_dtypes

import concourse.bass as bass
import concourse.bacc as bacc
import concourse.tile as tile
from concourse import mybir
from concourse.bass_utils import run_bass_kernel_spmd

B, N, D, NH, HDIM, FF = 256, 128, 256, 8, 32, 1024
NCORES = 8
PB = B // NCORES  # molecules per core

F32 = mybir.dt.float32
BF16 = mybir.dt.bfloat16
AF = mybir.ActivationFunctionType
AX = mybir.AxisListType
OP = mybir.AluOpType

_BUILT = None


def _ln_addnorm(nc, work, stats, y, g_rep, b_rep, resid, tag, eps):
    """out = resid + LN(y) * g + b, all [128, 256] f32 tiles."""
    s = stats.tile([N, 1], F32, name=f"s_{tag}", tag="ln_s")
    nc.vector.tensor_reduce(s, y, axis=AX.X, op=OP.add)
    mean = stats.tile([N, 1], F32, name=f"mean_{tag}", tag="ln_mean")
    nc.scalar.activation(mean, s, AF.Copy, scale=1.0 / D)
    xc = work.tile([N, D], F32, name=f"xc_{tag}", tag=f"xc_{tag}")
    nc.vector.tensor_scalar_sub(xc, y, mean)
    sq = work.tile([N, D], F32, name=f"sq_{tag}", tag=f"sq_{tag}")
    nc.vector.tensor_mul(sq, xc, xc)
    vs = stats.tile([N, 1], F32, name=f"vs_{tag}", tag="ln_vs")
    nc.vector.tensor_reduce(vs, sq, axis=AX.X, op=OP.add)
    sd = stats.tile([N, 1], F32, name=f"sd_{tag}", tag="ln_sd")
    nc.scalar.activation(sd, vs, AF.Sqrt, scale=1.0 / D, bias=eps)
    rstd = stats.tile([N, 1], F32, name=f"rstd_{tag}", tag="ln_rstd")
    nc.vector.reciprocal(rstd, sd)
    t = work.tile([N, D], F32, name=f"t_{tag}", tag=f"t_{tag}")
    nc.scalar.activation(t, xc, AF.Copy, scale=rstd)
    nc.vector.tensor_mul(t, t, g_rep)
    nc.vector.tensor_add(t, t, b_rep)
    xo = work.tile([N, D], F32, name=f"xo_{tag}", tag=f"xo_{tag}")
    nc.vector.tensor_add(xo, t, resid)
    return xo


def _build():
    global _BUILT
    if _BUILT is not None:
        return _BUILT

    nc = bacc.Bacc("TRN2", target_bir_lowering=False, debug=False)

    d_x = nc.dram_tensor("x", [PB, N, D], F32, kind="ExternalInput").ap()
    d_bias = nc.dram_tensor("bias", [PB, N, N], F32, kind="ExternalInput").ap()
    d_wq = nc.dram_tensor("wq", [D, D], BF16, kind="ExternalInput").ap()
    d_wk = nc.dram_tensor("wk", [D, D], BF16, kind="ExternalInput").ap()
    d_wv = nc.dram_tensor("wv", [D, D], BF16, kind="ExternalInput").ap()
    d_wo = nc.dram_tensor("wo", [D, D], BF16, kind="ExternalInput").ap()
    d_w1 = nc.dram_tensor("w1", [D, FF], BF16, kind="ExternalInput").ap()
    d_w2 = nc.dram_tensor("w2", [FF, D], BF16, kind="ExternalInput").ap()
    d_bq = nc.dram_tensor("bqc", [N, 2], F32, kind="ExternalInput").ap()
    d_bk = nc.dram_tensor("bkc", [N, 2], F32, kind="ExternalInput").ap()
    d_b1 = nc.dram_tensor("b1c", [N, NH], F32, kind="ExternalInput").ap()
    d_bo_rep = nc.dram_tensor("bo_rep", [N, D], F32, kind="ExternalInput").ap()
    d_b2_rep = nc.dram_tensor("b2_rep", [N, D], F32, kind="ExternalInput").ap()
    d_g1_rep = nc.dram_tensor("g1_rep", [N, D], F32, kind="ExternalInput").ap()
    d_h1_rep = nc.dram_tensor("h1_rep", [N, D], F32, kind="ExternalInput").ap()
    d_g2_rep = nc.dram_tensor("g2_rep", [N, D], F32, kind="ExternalInput").ap()
    d_h2_rep = nc.dram_tensor("h2_rep", [N, D], F32, kind="ExternalInput").ap()
    d_ident = nc.dram_tensor("ident", [N, N], BF16, kind="ExternalInput").ap()

    d_xout = nc.dram_tensor("x_out", [PB, N, D], F32, kind="ExternalOutput").ap()
    d_attn = nc.dram_tensor("attn_out", [PB, NH, N, N], F32, kind="ExternalOutput").ap()

    with tile.TileContext(nc) as tc:
        with (
            tc.tile_pool(name="wpool", bufs=1) as wp,
            tc.tile_pool(name="io", bufs=3) as io,
            tc.tile_pool(name="work", bufs=2) as work,
            tc.tile_pool(name="stats", bufs=4) as stats,
            tc.tile_pool(name="ps_sq", bufs=3, space="PSUM") as ps_sq,
            tc.tile_pool(name="ps_wide", bufs=2, space="PSUM") as ps_wide,
            tc.tile_pool(name="ps_tp", bufs=3, space="PSUM") as ps_tp,
        ):
            # ---- one-time weight loads ----
            wq_sb = wp.tile([N, 2, D], BF16)
            wk_sb = wp.tile([N, 2, D], BF16)
            wv_sb = wp.tile([N, 2, D], BF16)
            wo_sb = wp.tile([N, 2, D], BF16)
            for kc in range(2):
                nc.sync.dma_start(wq_sb[:, kc, :], d_wq[kc * N:(kc + 1) * N, :])
                nc.sync.dma_start(wk_sb[:, kc, :], d_wk[kc * N:(kc + 1) * N, :])
                nc.sync.dma_start(wv_sb[:, kc, :], d_wv[kc * N:(kc + 1) * N, :])
                nc.sync.dma_start(wo_sb[:, kc, :], d_wo[kc * N:(kc + 1) * N, :])
            w1_sb = wp.tile([N, 2, FF], BF16)
            for kc in range(2):
                nc.sync.dma_start(w1_sb[:, kc, :], d_w1[kc * N:(kc + 1) * N, :])
            w2_sb = wp.tile([N, NH, D], BF16)
            for fc in range(NH):
                nc.sync.dma_start(w2_sb[:, fc, :], d_w2[fc * N:(fc + 1) * N, :])
            bq_sb = wp.tile([N, 2], F32)
            nc.sync.dma_start(bq_sb[:], d_bq[:])
            bk_sb = wp.tile([N, 2], F32)
            nc.sync.dma_start(bk_sb[:], d_bk[:])
            b1_sb = wp.tile([N, NH], F32)
            nc.sync.dma_start(b1_sb[:], d_b1[:])
            bo_rep = wp.tile([N, D], F32)
            nc.sync.dma_start(bo_rep[:], d_bo_rep[:])
            b2_rep = wp.tile([N, D], F32)
            nc.sync.dma_start(b2_rep[:], d_b2_rep[:])
            g1_rep = wp.tile([N, D], F32)
            nc.sync.dma_start(g1_rep[:], d_g1_rep[:])
            h1_rep = wp.tile([N, D], F32)
            nc.sync.dma_start(h1_rep[:], d_h1_rep[:])
            g2_rep = wp.tile([N, D], F32)
            nc.sync.dma_start(g2_rep[:], d_g2_rep[:])
            h2_rep = wp.tile([N, D], F32)
            nc.sync.dma_start(h2_rep[:], d_h2_rep[:])
            ident = wp.tile([N, N], BF16)
            nc.sync.dma_start(ident[:], d_ident[:])
            eps_sb = wp.tile([N, 1], F32)
            nc.gpsimd.memset(eps_sb[:], 1e-5)

            # ---- per-molecule pipeline ----
            for m in range(PB):
                x_in = io.tile([N, D], F32, name=f"x_in_{m}", tag="x_in")
                nc.sync.dma_start(x_in[:], d_x[m])
                bias_sb = io.tile([N, N], F32, name=f"bias_{m}", tag="bias")
                nc.sync.dma_start(bias_sb[:], d_bias[m])

                xbf = work.tile([N, D], BF16, name=f"xbf_{m}", tag="xbf")
                nc.scalar.activation(xbf, x_in, AF.Copy)
                xT = work.tile([N, 2, N], BF16, name=f"xT_{m}", tag="xT")
                for c in range(2):
                    pst = ps_tp.tile([N, N], BF16, name=f"xTp_{m}_{c}", tag="tp")
                    nc.tensor.transpose(pst, xbf[:, c * N:(c + 1) * N], ident)
                    nc.vector.tensor_copy(xT[:, c, :], pst)

                # Q^T, K^T in [d_out, nodes] layout; V natural [nodes, d]
                qt = work.tile([N, 2, N], BF16, name=f"qt_{m}", tag="qt")
                kt = work.tile([N, 2, N], BF16, name=f"kt_{m}", tag="kt")
                for (w_sb, b_sb, dst, nm) in ((wq_sb, bq_sb, qt, "q"), (wk_sb, bk_sb, kt, "k")):
                    for c in range(2):
                        ps = ps_sq.tile([N, N], F32, name=f"ps{nm}_{m}_{c}", tag="sq")
                        for kc in range(2):
                            nc.tensor.matmul(
                                ps, lhsT=w_sb[:, kc, c * N:(c + 1) * N], rhs=xT[:, kc, :],
                                start=(kc == 0), stop=(kc == 1))
                        nc.scalar.activation(dst[:, c, :], ps, AF.Identity, bias=b_sb[:, c:c + 1])
                psv = ps_wide.tile([N, D], F32, name=f"psv_{m}", tag="wide")
                for kc in range(2):
                    nc.tensor.matmul(psv, lhsT=xT[:, kc, :], rhs=wv_sb[:, kc, :],
                                     start=(kc == 0), stop=(kc == 1))
                vbf = work.tile([N, D], BF16, name=f"vbf_{m}", tag="vbf")
                nc.scalar.activation(vbf, psv, AF.Copy)

                # scores -> exp (no max-sub needed: scores bounded ~15) -> rowsum
                sexp = work.tile([N, NH, N], F32, name=f"sexp_{m}", tag="sexp")
                ssum = work.tile([N, NH], F32, name=f"ssum_{m}", tag="ssum")
                s_sb = work.tile([N, NH, N], F32, name=f"s_sb_{m}", tag="s_sb")
                for h in range(NH):
                    ps = ps_sq.tile([N, N], F32, name=f"pss_{m}_{h}", tag="sq")
                    r0, c0 = (h % 4) * HDIM, h // 4
                    nc.tensor.matmul(ps, lhsT=qt[r0:r0 + HDIM, c0, :], rhs=kt[r0:r0 + HDIM, c0, :],
                                     start=True, stop=True, tile_position=(r0, 0))
                    nc.vector.tensor_add(s_sb[:, h, :], ps, bias_sb)
                    nc.scalar.activation(sexp[:, h, :], s_sb[:, h, :], AF.Exp,
                                         accum_out=ssum[:, h:h + 1])
                rs = work.tile([N, NH], F32, name=f"rs_{m}", tag="rs")
                nc.vector.reciprocal(rs, ssum)

                attn_f = work.tile([N, NH, N], F32, name=f"attn_f_{m}", tag="attn_f")
                attn_b = work.tile([N, NH, N], BF16, name=f"attn_b_{m}", tag="attn_b")
                attnT = work.tile([N, NH, N], BF16, name=f"attnT_{m}", tag="attnT")
                for h in range(NH):
                    nc.vector.tensor_scalar_mul(attn_f[:, h, :], sexp[:, h, :], rs[:, h:h + 1])
                    nc.sync.dma_start(d_attn[m, h], attn_f[:, h, :])
                    nc.scalar.activation(attn_b[:, h, :], sexp[:, h, :], AF.Copy, scale=rs[:, h:h + 1])
                    pst = ps_tp.tile([N, N], BF16, name=f"aTp_{m}_{h}", tag="tp")
                    nc.tensor.transpose(pst, attn_b[:, h, :], ident)
                    nc.vector.tensor_copy(attnT[:, h, :], pst)

                # ctx^T[d, q] accumulated per 4-head group into one psum bank
                ctxT = work.tile([N, 2, N], BF16, name=f"ctxT_{m}", tag="ctxT")
                for c in range(2):
                    psc = ps_sq.tile([N, N], F32, name=f"psc_{m}_{c}", tag="sq")
                    for hh in range(4):
                        h = c * 4 + hh
                        nc.tensor.matmul(psc[hh * HDIM:(hh + 1) * HDIM, :],
                                         lhsT=vbf[:, h * HDIM:(h + 1) * HDIM],
                                         rhs=attnT[:, h, :], start=True, stop=True,
                                         tile_position=(0, hh * HDIM))
                    nc.scalar.activation(ctxT[:, c, :], psc, AF.Copy)

                psy = ps_wide.tile([N, D], F32, name=f"psy_{m}", tag="wide")
                for kc in range(2):
                    nc.tensor.matmul(psy, lhsT=ctxT[:, kc, :], rhs=wo_sb[:, kc, :],
                                     start=(kc == 0), stop=(kc == 1))
                y1 = work.tile([N, D], F32, name=f"y1_{m}", tag="y1")
                nc.vector.tensor_add(y1, psy, bo_rep)
                x1 = _ln_addnorm(nc, work, stats, y1, g1_rep, h1_rep, x_in, "ln1", eps_sb)

                # FFN
                x1b = work.tile([N, D], BF16, name=f"x1b_{m}", tag="x1b")
                nc.scalar.activation(x1b, x1, AF.Copy)
                x1T = work.tile([N, 2, N], BF16, name=f"x1T_{m}", tag="x1T")
                for c in range(2):
                    pst = ps_tp.tile([N, N], BF16, name=f"x1Tp_{m}_{c}", tag="tp")
                    nc.tensor.transpose(pst, x1b[:, c * N:(c + 1) * N], ident)
                    nc.vector.tensor_copy(x1T[:, c, :], pst)
                ht = work.tile([N, NH, N], BF16, name=f"ht_{m}", tag="ht")
                for fc in range(NH):
                    ps = ps_sq.tile([N, N], F32, name=f"psh_{m}_{fc}", tag="sq")
                    for kc in range(2):
                        nc.tensor.matmul(ps, lhsT=w1_sb[:, kc, fc * N:(fc + 1) * N],
                                         rhs=x1T[:, kc, :], start=(kc == 0), stop=(kc == 1))
                    nc.scalar.activation(ht[:, fc, :], ps, AF.Gelu_apprx_tanh, bias=b1_sb[:, fc:fc + 1])
                psy2 = ps_wide.tile([N, D], F32, name=f"psy2_{m}", tag="wide")
                for fc in range(NH):
                    nc.tensor.matmul(psy2, lhsT=ht[:, fc, :], rhs=w2_sb[:, fc, :],
                                     start=(fc == 0), stop=(fc == NH - 1))
                y2 = work.tile([N, D], F32, name=f"y2_{m}", tag="y2")
                nc.vector.tensor_add(y2, psy2, b2_rep)
                x2 = _ln_addnorm(nc, work, stats, y2, g2_rep, h2_rep, x1, "ln2", eps_sb)
                nc.sync.dma_start(d_xout[m], x2)

    nc.compile()
    _BUILT = nc
    return nc


def _prep_inputs(inputs):
    f32 = np.float32
    bf16 = ml_dtypes.bfloat16
    x = np.ascontiguousarray(inputs["node_embedding"], dtype=f32)
    mask = np.asarray(inputs["node_mask"], dtype=f32)
    bias = (np.asarray(inputs["adjacency"], dtype=f32)
            + np.asarray(inputs["distance"], dtype=f32)
            + ((1.0 - mask) * np.float32(-1e9))[:, None, :])
    bias = np.ascontiguousarray(bias, dtype=f32)

    sc = 1.0 / math.sqrt(HDIM)
    rep = lambda v: np.ascontiguousarray(np.broadcast_to(np.asarray(v, f32)[None, :], (N, D)))
    chunk = lambda v, k: np.ascontiguousarray(np.asarray(v, f32).reshape(k, N).T)
    wo = np.asarray(inputs["Wo"], f32)
    bo_eff = np.asarray(inputs["bv"], f32) @ wo + np.asarray(inputs["bo"], f32)
    common = {
        "wq": np.ascontiguousarray((np.asarray(inputs["Wq"], f32) * sc).astype(bf16)),
        "wk": np.ascontiguousarray(np.asarray(inputs["Wk"], f32).astype(bf16)),
        "wv": np.ascontiguousarray(np.asarray(inputs["Wv"], f32).astype(bf16)),
        "wo": np.ascontiguousarray(wo.astype(bf16)),
        "w1": np.ascontiguousarray(np.asarray(inputs["W1"], f32).astype(bf16)),
        "w2": np.ascontiguousarray(np.asarray(inputs["W2"], f32).astype(bf16)),
        "bqc": chunk(np.asarray(inputs["bq"], f32) * sc, 2),
        "bkc": chunk(inputs["bk"], 2),
        "b1c": chunk(inputs["b1"], NH),
        "bo_rep": rep(bo_eff),
        "b2_rep": rep(inputs["b2"]),
        "g1_rep": rep(inputs["ln1_g"]),
        "h1_rep": rep(inputs["ln1_b"]),
        "g2_rep": rep(inputs["ln2_g"]),
        "h2_rep": rep(inputs["ln2_b"]),
        "ident": np.eye(N, dtype=bf16),
    }
    in_maps = []
    for c in range(NCORES):
        sl = slice(c * PB, (c + 1) * PB)
        in_maps.append({"x": x[sl], "bias": bias[sl], **common})
    return in_maps


def kernel(**inputs):
    nc = _build()
    in_maps = _prep_inputs(inputs)
    res = run_bass_kernel_spmd(nc, in_maps, list(range(NCORES))).results
    x_out = np.concatenate([res[c]["x_out"] for c in range(NCORES)], axis=0)
    attn = np.concatenate([res[c]["attn_out"] for c in range(NCORES)], axis=0)
    return x_out.astype(np.float32), attn.astype(np.float32)
